# revision 1
# baseline (speedup 1.0000x reference)
"""Trainium2 Bass kernel for nn_Decoder (LSTM decoder: embed -> LSTM -> vocab fc).

Sharding: data-parallel over batch across 8 cores (16 rows each). No collectives.
Per core:
  - gates matmul uses 4-way PE column tiling (128x32 mode): 4 concurrent
    moving streams of the (interleaved-column) combined weight W_cat^T.
  - h is produced H-major via DVE 32x32 stream transposes, so it feeds the
    next step's stationary operand and the fc matmul directly.
  - fc (vocab) projection runs on full-array matmuls over 256-token blocks,
    interleaved with the recurrence to fill PE gaps.
All matmuls bf16 (fp32 psum accumulation); c-state kept fp32.
"""

import sys

sys.path.insert(0, "/opt/trn_rl_repo")

import numpy as np
import ml_dtypes

import concourse.bass as bass
import concourse.bacc as bacc
import concourse.mybir as mybir
import concourse.tile as tile

BF16 = ml_dtypes.bfloat16

# Problem shapes
B, T, E, H, V = 128, 64, 512, 1024, 10000
NCORES = 8
BC = B // NCORES  # 16 batch rows per core
G = 4             # column-tile groups
KZ = 13           # K chunks: 4 (E) + 8 (H) + 1 (bias)
VT = 79           # vocab tiles of 128 (V padded to 10112)
VP = VT * 128
VAUG = V + BC     # embed table + per-core feature rows
FC_BLOCK = 16     # steps per fc token block (256 tokens)
FC_PER_STEP = 5   # fc vocab tiles emitted per step once blocks are ready

F32 = mybir.dt.float32
BF = mybir.dt.bfloat16
I32 = mybir.dt.int32


def build_nc(t_steps=T, n_vt=VT, fc_block=FC_BLOCK, fc_per_step=FC_PER_STEP):
    nc = bacc.Bacc()

    embed_d = nc.declare_dram_parameter("embed", [VAUG, E], BF, isOutput=False)
    idx_d = nc.declare_dram_parameter("idx", [32, t_steps], I32, isOutput=False)
    wmov_d = nc.declare_dram_parameter("wmov", [G, KZ, 128, 1024], BF, isOutput=False)
    ones_d = nc.declare_dram_parameter("onespad", [128, BC], BF, isOutput=False)
    sel_d = nc.declare_dram_parameter("sel16", [128, BC], BF, isOutput=False)
    xg_d = nc.dram_tensor("xg_scratch", [t_steps, BC, G, 1024], BF)
    fcw_d = nc.declare_dram_parameter("fcw", [n_vt, 128, 8, 128], BF, isOutput=False)
    fcbt_d = nc.declare_dram_parameter("fcbt", [128, n_vt], F32, isOutput=False)
    ntok = t_steps * BC
    out_d = nc.declare_dram_parameter("out_lg", [n_vt, 128, ntok], F32, isOutput=True)

    blk_tok = fc_block * BC  # tokens per fc block

    with tile.TileContext(nc) as tc:
        with (
            tc.tile_pool(name="persist", bufs=1) as pp,
            tc.tile_pool(name="xsraw", bufs=8) as xsraw_p,
            tc.tile_pool(name="xst", bufs=8) as xst_p,
            tc.tile_pool(name="gates", bufs=3) as gates_p,
            tc.tile_pool(name="ew", bufs=3) as ew_p,
            tc.tile_pool(name="fcw", bufs=6) as fcw_p,
            tc.tile_pool(name="logit", bufs=6) as logit_p,
            tc.tile_pool(name="gpsum", bufs=1, space="PSUM") as gps_p,
            tc.tile_pool(name="fpsum", bufs=3, space="PSUM") as fps_p,
        ):
            # ---- persistent tiles + prologue loads ----
            wsb = {}
            for k in [0, 1, 2, 3, 12] + list(range(4, 12)):
                for g in range(G):
                    w = pp.tile([128, 1024], BF, tag=f"w_{g}_{k}", name=f"w_{g}_{k}")
                    nc.sync.dma_start(out=w[:, :], in_=wmov_d[g, k, :, :])
                    wsb[(g, k)] = w
            idx_sb = pp.tile([32, t_steps], I32, tag="idx")
            nc.sync.dma_start(out=idx_sb[:, :], in_=idx_d[:, :])
            ones_sb = pp.tile([128, BC], BF, tag="ones")
            nc.sync.dma_start(out=ones_sb[:, :], in_=ones_d[:, :])
            sel_sb = pp.tile([128, BC], BF, tag="sel16")
            nc.sync.dma_start(out=sel_sb[:, :], in_=sel_d[:, :])
            # persistent double-buffered xg moving tiles; rows >= BC stay zero
            xgm_sb = {}
            for g in range(G):
                for par in range(2):
                    tname = f"xgm_{g}_{par}"
                    xt = pp.tile([128, 1024], BF, tag=tname, name=tname)
                    nc.vector.memset(xt[:, :], 0.0)
                    xgm_sb[(g, par)] = xt
            fcbt_sb = pp.tile([128, n_vt], F32, tag="fcbt")
            nc.sync.dma_start(out=fcbt_sb[:, :], in_=fcbt_d[:, :])

            hsT = []
            for kc in range(8):
                hsT.append(
                    pp.tile([128, ntok + 32], BF, tag=f"hsT_{kc}", name=f"hsT_{kc}")
                )
            c_sb = pp.tile([128, 256], F32, tag="c_state")

            gps = [
                gps_p.tile([128, 1024], F32, tag="gps0", name="gps0"),
                gps_p.tile([128, 1024], F32, tag="gps1", name="gps1"),
            ]
            nc.vector.memset(gps[0][:, :], 0.0)
            nc.vector.memset(gps[1][:, :], 0.0)

            # ---- embedding gathers + xs transposes (token-column layout) ----
            xsT = []
            for ec in range(4):
                xsT.append(
                    pp.tile([128, ntok + 32], BF, tag=f"xsT_{ec}", name=f"xsT_{ec}")
                )
            for t in range(t_steps):
                xsr = xsraw_p.tile([32, E], BF, tag="xsr")
                nc.gpsimd.indirect_dma_start(
                    out=xsr[:, :],
                    out_offset=None,
                    in_=embed_d[:, :],
                    in_offset=bass.IndirectOffsetOnAxis(
                        ap=idx_sb[:, t : t + 1], axis=0
                    ),
                )
                for ec in range(4):
                    for beta in range(4):
                        nc.vector.transpose(
                            out=xsT[ec][
                                32 * beta : 32 * beta + 32, BC * t : BC * t + 32
                            ],
                            in_=xsr[0:32, 128 * ec + 32 * beta : 128 * ec + 32 * beta + 32],
                        )

            # ---- xg precompute: xg = xs @ W_ih^T at full-array efficiency ----
            # out per (token-tile, group): [128 tokens, 1024 cols] in gps psum,
            # evicted bf16 to DRAM scratch [t, b, g, 1024]
            for mt in range(t_steps * BC // 128):
                for gpair in ((0, 1), (2, 3)):
                    for gi, g in enumerate(gpair):
                        xps = gps[gi]
                        for ec in range(4):
                            for half in range(2):
                                cs = slice(512 * half, 512 * half + 512)
                                nc.tensor.matmul(
                                    xps[:, cs],
                                    xsT[ec][:, 128 * mt : 128 * mt + 128],
                                    wsb[(g, ec)][:, cs],
                                    start=(ec == 0),
                                    stop=(ec == 3),
                                )
                        xev = logit_p.tile([128, 1024], BF, tag="xev")
                        nc.vector.tensor_copy(xev[:, :], xps[:, :])
                        nc.sync.dma_start(
                            out=xg_d[8 * mt : 8 * mt + 8, :, g, :],
                            in_=xev[:, :],
                        )
            # gates psum junk rows must be zero for the EW partition-span trick
            nc.vector.memset(gps[0][:, :], 0.0)
            nc.vector.memset(gps[1][:, :], 0.0)

            # ---- fc emission helper ----
            fc_queue = []  # (block_idx, vtile)
            n_blocks = t_steps // fc_block

            def emit_fc(n):
                for _ in range(min(n, len(fc_queue))):
                    kblk, v = fc_queue.pop(0)
                    fcw_t = fcw_p.tile([128, 1024], BF, tag="fcw_t")
                    nc.sync.dma_start(out=fcw_t[:, :], in_=fcw_d[v, :, :, :])
                    fps = fps_p.tile([128, blk_tok], F32, tag="fps")
                    for kc in range(8):
                        nc.tensor.matmul(
                            fps[:, :],
                            fcw_t[:, 128 * kc : 128 * kc + 128],
                            hsT[kc][:, blk_tok * kblk : blk_tok * (kblk + 1)],
                            start=(kc == 0),
                            stop=(kc == 7),
                        )
                    lg = logit_p.tile([128, blk_tok], F32, tag="lg")
                    nc.scalar.activation(
                        lg[:, :],
                        fps[:, :],
                        mybir.ActivationFunctionType.Identity,
                        bias=fcbt_sb[:, v : v + 1],
                    )
                    nc.sync.dma_start(
                        out=out_d[v, :, blk_tok * kblk : blk_tok * (kblk + 1)],
                        in_=lg[:, :],
                    )

            # ---- recurrence ----
            for t in range(t_steps):
                ps = gps[t % 2]
                for g in range(G):
                    nc.sync.dma_start(
                        out=xgm_sb[(g, t % 2)][0:BC, :], in_=xg_d[t, :, g, :]
                    )
                # h-independent chunks first so next-step PE work overlaps
                # this step's EW/transpose tail. k=13 -> precomputed xg chunk.
                ks = [13, 12] if t == 0 else [13, 12] + list(range(4, 12))
                for ki, k in enumerate(ks):
                    for half in range(2):
                        cs = slice(512 * half, 512 * half + 512)
                        for g in range(G):
                            if k == 13:
                                stat = sel_sb[:, :]
                                mov = xgm_sb[(g, t % 2)][:, cs]
                            elif k == 12:
                                stat = ones_sb[:, :]
                                mov = wsb[(g, 12)][:, cs]
                            else:
                                stat = hsT[k - 4][:, BC * (t - 1) : BC * (t - 1) + BC]
                                mov = wsb[(g, k)][:, cs]
                            nc.tensor.matmul(
                                ps[32 * g : 32 * g + BC, cs],
                                stat,
                                mov,
                                start=(ki == 0),
                                stop=(ki == len(ks) - 1),
                                tile_position=(0, 32 * g),
                                skip_group_check=True,
                            )

                gt = gates_p.tile([128, 1024], F32, tag="gt")
                nc.scalar.activation(
                    gt[:, 0:768], ps[:, 0:768], mybir.ActivationFunctionType.Sigmoid
                )
                nc.scalar.activation(
                    gt[:, 768:1024], ps[:, 768:1024], mybir.ActivationFunctionType.Tanh
                )
                # c = f*c + i*g ; h = o*tanh(c)
                if t == 0:
                    nc.vector.tensor_mul(c_sb[:, :], gt[:, 0:256], gt[:, 768:1024])
                else:
                    tmp1 = ew_p.tile([128, 256], F32, tag="tmp1")
                    nc.vector.tensor_mul(tmp1[:, :], gt[:, 0:256], gt[:, 768:1024])
                    nc.vector.tensor_mul(c_sb[:, :], gt[:, 256:512], c_sb[:, :])
                    nc.vector.tensor_add(c_sb[:, :], c_sb[:, :], tmp1[:, :])
                tcs = ew_p.tile([128, 256], F32, tag="tcs")
                nc.scalar.activation(
                    tcs[:, :], c_sb[:, :], mybir.ActivationFunctionType.Tanh
                )
                h_sb = ew_p.tile([128, 256], BF, tag="h_sb")
                nc.vector.tensor_mul(h_sb[:, :], gt[:, 512:768], tcs[:, :])

                # h -> hsT (H-major), 32x32 blocks
                for g in range(G):
                    for gam in range(2):
                        kc = 2 * g + gam
                        for beta in range(4):
                            nc.vector.transpose(
                                out=hsT[kc][
                                    32 * beta : 32 * beta + 32,
                                    BC * t : BC * t + 32,
                                ],
                                in_=h_sb[
                                    32 * g : 32 * g + 32,
                                    128 * gam + 32 * beta : 128 * gam + 32 * beta + 32,
                                ],
                            )

                # queue fc work for completed blocks; interleave emission
                if (t + 1) % fc_block == 0:
                    kblk = (t + 1) // fc_block - 1
                    for v in range(n_vt):
                        fc_queue.append((kblk, v))
                if t >= fc_block:
                    emit_fc(fc_per_step)

            emit_fc(len(fc_queue))

    nc.finalize()
    return nc


def prep_host(features, captions, embed_W, W_ih, W_hh, b_ih, b_hh, fc_W, fc_b,
              t_steps=T, n_vt=VT):
    """Host-side layout prep. Returns (shared dict, per-core lists)."""
    # gate-column permutation: group g holds H-range [256g:256g+256) of each
    # gate, column order within group = [i | f | o | gg] (256 each)
    sec_base = np.array([0, H, 3 * H, 2 * H])
    j = np.arange(1024)
    perm = np.empty((G, 1024), np.int64)
    for g in range(G):
        perm[g] = sec_base[j // 256] + 256 * g + (j % 256)

    W_cat = np.concatenate([W_ih, W_hh], axis=1)  # [4H, E+H]
    bias = (b_ih + b_hh).astype(np.float32)

    wmov = np.zeros((G, KZ, 128, 1024), np.float32)
    for g in range(G):
        sel = W_cat[perm[g]]  # [1024 cols, 1536]
        for k in range(12):
            wmov[g, k] = sel[:, 128 * k : 128 * k + 128].T
        wmov[g, 12, 0, :] = bias[perm[g]]
    wmov = wmov.astype(BF16)

    onespad = np.zeros((128, BC), np.float32)
    onespad[0, :] = 1.0
    onespad = onespad.astype(BF16)

    sel16 = np.zeros((128, BC), np.float32)
    sel16[:BC, :BC] = np.eye(BC)
    sel16 = sel16.astype(BF16)

    vp = n_vt * 128
    nv = min(V, vp)
    fc_W_pad = np.zeros((vp, H), np.float32)
    fc_W_pad[:nv] = fc_W[:nv]
    fcw = np.ascontiguousarray(
        fc_W_pad.T.reshape(8, 128, n_vt, 128).transpose(2, 1, 0, 3)
    ).astype(BF16)  # [v, p, kc, j]

    fc_b_pad = np.zeros((vp,), np.float32)
    fc_b_pad[:nv] = fc_b[:nv]
    fcbt = np.ascontiguousarray(fc_b_pad.reshape(n_vt, 128).T).astype(np.float32)

    shared = {"wmov": wmov, "onespad": onespad, "sel16": sel16, "fcw": fcw,
              "fcbt": fcbt}

    per_core = []
    for c in range(NCORES):
        rows = slice(c * BC, (c + 1) * BC)
        emb = np.concatenate(
            [embed_W.astype(np.float32), features[rows].astype(np.float32)], axis=0
        ).astype(BF16)
        idx = np.zeros((32, t_steps), np.int32)
        idx[:BC, 0] = V + np.arange(BC)
        if t_steps > 1:
            idx[:BC, 1:] = captions[rows, 1:t_steps].astype(np.int32)
        per_core.append({"embed": emb, "idx": idx})
    return shared, per_core


_NC_CACHE = {}


def kernel(features, captions, embed_W, W_ih, W_hh, b_ih, b_hh, fc_W, fc_b):
    from concourse.bass_utils import run_bass_kernel_spmd

    features = np.asarray(features)
    captions = np.asarray(captions)
    embed_W = np.asarray(embed_W)
    W_ih = np.asarray(W_ih)
    W_hh = np.asarray(W_hh)
    b_ih = np.asarray(b_ih)
    b_hh = np.asarray(b_hh)
    fc_W = np.asarray(fc_W)
    fc_b = np.asarray(fc_b)

    if "nc" not in _NC_CACHE:
        _NC_CACHE["nc"] = build_nc()
    nc = _NC_CACHE["nc"]

    shared, per_core = prep_host(
        features, captions, embed_W, W_ih, W_hh, b_ih, b_hh, fc_W, fc_b
    )
    in_maps = [{**shared, **pc} for pc in per_core]
    res = run_bass_kernel_spmd(nc, in_maps, list(range(NCORES)))
    _NC_CACHE["last_results"] = res
    _NC_CACHE["last_in_maps"] = in_maps

    out = np.empty((B, T, V), np.float32)
    ntok = T * BC
    for c in range(NCORES):
        lg = res.results[c]["out_lg"].reshape(VP, ntok)[:V]  # [V, T*BC]
        out[c * BC : (c + 1) * BC] = lg.reshape(V, T, BC).transpose(2, 1, 0)
    return out



# revision 3
# speedup vs baseline: 2.0819x; 2.0819x over previous
"""Trainium2 Bass kernel for nn_Decoder (LSTM decoder: embed -> LSTM -> vocab fc).

Strategy (v2):
  - Host folds embedding + input projection + biases into one gather table:
    xg_table = embed_W @ W_ih^T + b_ih + b_hh  (gate-column-permuted). Per
    step the kernel indirect-DMA-gathers 16 rows -> no device-side embedding
    transposes, no xg matmuls, no bias matmuls.
  - Recurrence is data-parallel over batch (16 rows/core), 4-way PE column
    tiling for the h @ W_hh^T matmul (as before). h^T is produced by 2 PE
    transposes + 8 narrow DVE copies (instead of 32 DVE transposes).
  - Every S=8 steps, the per-core h^T block is AllGather'd (DRAM->DRAM) so
    every core holds h^T for the FULL batch; the fc projection is sharded
    over vocab (1280 rows/core) with its weights RESIDENT in SBUF, and its
    matmuls (full-array, N=512) are interleaved into the recurrence's PE
    gaps.
"""

import sys

sys.path.insert(0, "/opt/trn_rl_repo")

import numpy as np
import ml_dtypes

import concourse.bass as bass
import concourse.bacc as bacc
import concourse.mybir as mybir
import concourse.tile as tile

BF16 = ml_dtypes.bfloat16

# Problem shapes
B, T, E, H, V = 128, 64, 512, 1024, 10000
NCORES = 8
BC = B // NCORES        # 16 batch rows per core
G = 4                   # PE column-tile groups for the recurrence
S = 8                   # steps per h^T block (AllGather granularity)
NB = T // S             # 8 blocks
VC = 1280               # vocab rows per core (10240 padded / 8)
NVT = VC // 128         # 10 vocab tiles per core
VAUG = V + BC           # xg table rows: vocab + per-core feature rows
LAG = 3                 # steps between block boundary and fc eligibility
FC_A = 2                # fc units emitted after the h-matmuls
FC_B = 2                # fc units emitted after the transposes

F32 = mybir.dt.float32
BF = mybir.dt.bfloat16
I32 = mybir.dt.int32


def build_nc():
    nc = bacc.Bacc("TRN2", num_devices=NCORES)

    xgt_d = nc.declare_dram_parameter("xgt", [VAUG, 4 * H], BF, isOutput=False)
    idx_d = nc.declare_dram_parameter("idx", [BC, T], I32, isOutput=False)
    whh_d = nc.declare_dram_parameter("whh", [G, 8, 128, 1024], BF, isOutput=False)
    sel_d = nc.declare_dram_parameter("sel16", [128, BC], BF, isOutput=False)
    id_d = nc.declare_dram_parameter("ident", [128, 128], BF, isOutput=False)
    fcw_d = nc.declare_dram_parameter("fcw", [NVT, 8, 128, 128], BF, isOutput=False)
    fcb_d = nc.declare_dram_parameter("fcb", [128, NVT], F32, isOutput=False)
    # out layout: (vtile, vpart, src_core, block, s*16+j)
    out_d = nc.declare_dram_parameter(
        "out_lg", [NVT, 128, NCORES, NB, S * BC], F32, isOutput=True
    )

    hsb_in_d = nc.dram_tensor("hsb_in", [NB, 8, 128, S * BC], BF)
    hsb_out_d = nc.dram_tensor(
        "hsb_out", [NB, NCORES, 8, 128, S * BC], BF, addr_space="Shared"
    )

    XB = 4  # xgm prefetch depth

    with tile.TileContext(nc) as tc:
        with (
            tc.tile_pool(name="persist", bufs=1) as pp,
            tc.tile_pool(name="gates", bufs=3) as gates_p,
            tc.tile_pool(name="ew", bufs=3) as ew_p,
            tc.tile_pool(name="logit", bufs=4) as logit_p,
            tc.tile_pool(name="gpsum", bufs=1, space="PSUM") as gps_p,
            tc.tile_pool(name="fpsum", bufs=2, space="PSUM") as fps_p,
            tc.tile_pool(name="tpsum", bufs=2, space="PSUM") as tps_p,
        ):
            # ---- small persistent tiles first (cheap DMAs, unblock step 0) ----
            idx_sb = pp.tile([BC, T], I32, tag="idx")
            nc.sync.dma_start(out=idx_sb[:, :], in_=idx_d[:, :])
            sel_sb = pp.tile([128, BC], BF, tag="sel16")
            nc.sync.dma_start(out=sel_sb[:, :], in_=sel_d[:, :])
            id_sb = pp.tile([128, 128], BF, tag="ident")
            nc.sync.dma_start(out=id_sb[:, :], in_=id_d[:, :])
            fcb_sb = pp.tile([128, NVT], F32, tag="fcb")
            nc.sync.dma_start(out=fcb_sb[:, :], in_=fcb_d[:, :])

            # xgm ring: rows 0:16 hold gathered xg rows; junk rows stay 0
            xgm = []
            for par in range(XB):
                xt = pp.tile([128, 4 * H], BF, tag=f"xgm_{par}", name=f"xgm_{par}")
                nc.vector.memset(xt[:, :], 0.0)
                xgm.append(xt)

            def gather_xg(t):
                nc.gpsimd.indirect_dma_start(
                    out=xgm[t % XB][0:BC, :],
                    out_offset=None,
                    in_=xgt_d[:, :],
                    in_offset=bass.IndirectOffsetOnAxis(
                        ap=idx_sb[:, t : t + 1], axis=0
                    ),
                )

            for t in range(min(XB - 1, T)):
                gather_xg(t)

            # ---- weights ----
            whh_sb = {}
            for k in range(8):
                for g in range(G):
                    w = pp.tile([128, 1024], BF, tag=f"w_{g}_{k}", name=f"w_{g}_{k}")
                    nc.sync.dma_start(out=w[:, :], in_=whh_d[g, k, :, :])
                    whh_sb[(g, k)] = w
            fcw_sb = []
            for v in range(NVT):
                fw = pp.tile([128, 1024], BF, tag=f"fcw_{v}", name=f"fcw_{v}")
                for kc in range(8):
                    nc.sync.dma_start(
                        out=fw[:, 128 * kc : 128 * kc + 128], in_=fcw_d[v, kc, :, :]
                    )
                fcw_sb.append(fw)

            # ---- state tiles ----
            # h^T accumulation ring: 2 block slots of S*BC=128 cols per chunk
            hsT = [
                pp.tile([128, 2 * S * BC], BF, tag=f"hsT_{kc}", name=f"hsT_{kc}")
                for kc in range(8)
            ]
            # gathered full-batch h^T ring: 2 block slots
            hfull = [
                [
                    pp.tile([128, S * B], BF, tag=f"hf_{sl}_{kc}", name=f"hf_{sl}_{kc}")
                    for kc in range(8)
                ]
                for sl in range(2)
            ]
            c_sb = pp.tile([128, 256], F32, tag="c_state")

            gps = [
                gps_p.tile([128, 1024], F32, tag="gps0", name="gps0"),
                gps_p.tile([128, 1024], F32, tag="gps1", name="gps1"),
            ]
            nc.vector.memset(gps[0][:, :], 0.0)
            nc.vector.memset(gps[1][:, :], 0.0)

            # ---- fc emission ----
            fc_queue = []  # (block, vtile, half) eligible units

            def emit_fc(n):
                for _ in range(min(n, len(fc_queue))):
                    b, v, hf = fc_queue.pop(0)
                    sl = b % 2
                    fps = fps_p.tile([128, 512], F32, tag="fps")
                    for kc in range(8):
                        nc.tensor.matmul(
                            fps[:, :],
                            fcw_sb[v][:, 128 * kc : 128 * kc + 128],
                            hfull[sl][kc][:, 512 * hf : 512 * hf + 512],
                            start=(kc == 0),
                            stop=(kc == 7),
                        )
                    lg = logit_p.tile([128, 512], F32, tag="lg")
                    nc.scalar.activation(
                        lg[:, :],
                        fps[:, :],
                        mybir.ActivationFunctionType.Identity,
                        bias=fcb_sb[:, v : v + 1],
                    )
                    nc.sync.dma_start(
                        out=out_d[v, :, 4 * hf : 4 * hf + 4, b, :], in_=lg[:, :]
                    )

            pending = []  # blocks gathered but not yet eligible: (block, ready_t)

            def release_pending(t):
                while pending and pending[0][1] <= t:
                    b, _ = pending.pop(0)
                    for v in range(NVT):
                        for hf in range(2):
                            fc_queue.append((b, v, hf))

            # ---- recurrence ----
            for t in range(T):
                ps = gps[t % 2]
                release_pending(t)
                if t + XB - 1 < T:
                    gather_xg(t + XB - 1)

                # gate matmuls: xg injection first (independent of h(t-1)),
                # then the 8 h-chunk contributions
                nks = 1 if t == 0 else 9
                for half in range(2):
                    cs = slice(512 * half, 512 * half + 512)
                    for g in range(G):
                        nc.tensor.matmul(
                            ps[32 * g : 32 * g + BC, cs],
                            sel_sb[:, :],
                            xgm[t % XB][:, 1024 * g + 512 * half :][:, 0:512],
                            start=True,
                            stop=(nks == 1),
                            tile_position=(0, 32 * g),
                            skip_group_check=True,
                        )
                if t > 0:
                    pc = ((t - 1) // S) % 2 * (S * BC) + ((t - 1) % S) * BC
                    for ki in range(8):
                        for half in range(2):
                            cs = slice(512 * half, 512 * half + 512)
                            for g in range(G):
                                nc.tensor.matmul(
                                    ps[32 * g : 32 * g + BC, cs],
                                    hsT[ki][:, pc : pc + BC],
                                    whh_sb[(g, ki)][:, cs],
                                    start=False,
                                    stop=(ki == 7),
                                    tile_position=(0, 32 * g),
                                    skip_group_check=True,
                                )

                emit_fc(FC_A)

                # elementwise: gate order per group is [i | f | o | g]
                gt = gates_p.tile([128, 1024], F32, tag="gt")
                nc.scalar.activation(
                    gt[:, 0:768], ps[:, 0:768], mybir.ActivationFunctionType.Sigmoid
                )
                nc.scalar.activation(
                    gt[:, 768:1024], ps[:, 768:1024], mybir.ActivationFunctionType.Tanh
                )
                if t == 0:
                    nc.vector.tensor_mul(c_sb[:, :], gt[:, 0:256], gt[:, 768:1024])
                else:
                    tmp1 = ew_p.tile([128, 256], F32, tag="tmp1")
                    nc.vector.tensor_mul(tmp1[:, :], gt[:, 0:256], gt[:, 768:1024])
                    nc.vector.tensor_mul(c_sb[:, :], gt[:, 256:512], c_sb[:, :])
                    nc.vector.tensor_add(c_sb[:, :], c_sb[:, :], tmp1[:, :])
                tcs = ew_p.tile([128, 256], F32, tag="tcs")
                nc.scalar.activation(
                    tcs[:, :], c_sb[:, :], mybir.ActivationFunctionType.Tanh
                )
                h_sb = ew_p.tile([128, 256], BF, tag="h_sb")
                nc.vector.tensor_mul(h_sb[:, :], gt[:, 512:768], tcs[:, :])

                # h -> h^T: 2 PE transposes + 8 narrow copies
                cc = (t // S) % 2 * (S * BC) + (t % S) * BC
                for gam in range(2):
                    tps = tps_p.tile([128, 128], BF, tag="tps")
                    nc.tensor.transpose(
                        tps[:, :], h_sb[:, 128 * gam : 128 * gam + 128], id_sb[:, :]
                    )
                    for g in range(G):
                        nc.vector.tensor_copy(
                            hsT[2 * g + gam][:, cc : cc + BC],
                            tps[:, 32 * g : 32 * g + BC],
                        )

                emit_fc(FC_B)

                # block boundary: stage own h^T block, AllGather, load gathered
                if (t + 1) % S == 0:
                    b = t // S
                    sl = b % 2
                    for kc in range(8):
                        nc.sync.dma_start(
                            out=hsb_in_d[b, kc, :, :],
                            in_=hsT[kc][:, sl * S * BC : (sl + 1) * S * BC],
                        )
                    nc.gpsimd.collective_compute(
                        "AllGather",
                        mybir.AluOpType.bypass,
                        replica_groups=[list(range(NCORES))],
                        ins=[hsb_in_d[b, :, :, :]],
                        outs=[hsb_out_d[b, :, :, :, :]],
                    )
                    for core in range(NCORES):
                        for kc in range(8):
                            nc.gpsimd.dma_start(
                                out=hfull[sl][kc][:, 128 * core : 128 * core + 128],
                                in_=hsb_out_d[b, core, kc, :, :],
                            )
                    pending.append((b, t + 1 + LAG))

            # ---- epilogue: drain remaining fc work ----
            release_pending(10**9)
            emit_fc(len(fc_queue))

    nc.finalize()
    return nc


def prep_host(features, captions, embed_W, W_ih, W_hh, b_ih, b_hh, fc_W, fc_b):
    """Host-side layout prep. Returns (shared dict, per-core list)."""
    # gate-column permutation: group g holds H-range [256g:256g+256) of each
    # gate, column order within group = [i | f | o | gg] (256 each)
    sec_base = np.array([0, H, 3 * H, 2 * H])
    j = np.arange(1024)
    perm = np.empty((G, 1024), np.int64)
    for g in range(G):
        perm[g] = sec_base[j // 256] + 256 * g + (j % 256)
    full_perm = perm.reshape(-1)  # [4096] column order: group-major

    bias = (b_ih + b_hh).astype(np.float32)

    # xg gather table: (embed @ W_ih^T + bias), columns permuted
    xgt_core = (embed_W.astype(np.float32) @ W_ih.T.astype(np.float32)) + bias
    xgt_core = xgt_core[:, full_perm].astype(BF16)  # [V, 4096]
    feat_xg = (features.astype(np.float32) @ W_ih.T.astype(np.float32)) + bias
    feat_xg = feat_xg[:, full_perm].astype(BF16)  # [B, 4096]

    whh = np.zeros((G, 8, 128, 1024), np.float32)
    for g in range(G):
        selw = W_hh[perm[g]]  # [1024 gate-cols, 1024]
        for k in range(8):
            whh[g, k] = selw[:, 128 * k : 128 * k + 128].T
    whh = whh.astype(BF16)

    sel16 = np.zeros((128, BC), np.float32)
    sel16[:BC, :BC] = np.eye(BC)
    sel16 = sel16.astype(BF16)
    ident = np.eye(128, dtype=np.float32).astype(BF16)

    vp = NCORES * VC  # 10240
    fc_W_pad = np.zeros((vp, H), np.float32)
    fc_W_pad[:V] = fc_W
    fc_b_pad = np.zeros((vp,), np.float32)
    fc_b_pad[:V] = fc_b

    shared = {"whh": whh, "sel16": sel16, "ident": ident}

    per_core = []
    for c in range(NCORES):
        rows = slice(c * BC, (c + 1) * BC)
        xgt = np.concatenate([xgt_core, feat_xg[rows]], axis=0)  # [VAUG, 4096]
        idx = np.zeros((BC, T), np.int32)
        idx[:, 0] = V + np.arange(BC)
        idx[:, 1:] = captions[rows, 1:T].astype(np.int32)
        wslice = fc_W_pad[c * VC : (c + 1) * VC]  # [1280, 1024]
        fcw = np.ascontiguousarray(
            wslice.reshape(NVT, 128, 8, 128).transpose(0, 2, 3, 1)
        ).astype(BF16)  # [v, kc, k, j]
        fcb = np.ascontiguousarray(
            fc_b_pad[c * VC : (c + 1) * VC].reshape(NVT, 128).T
        ).astype(np.float32)  # [128, NVT]
        per_core.append({"xgt": xgt, "idx": idx, "fcw": fcw, "fcb": fcb})
    return shared, per_core


_NC_CACHE = {}


def kernel(features, captions, embed_W, W_ih, W_hh, b_ih, b_hh, fc_W, fc_b):
    from concourse.bass_utils import run_bass_kernel_spmd

    features = np.asarray(features)
    captions = np.asarray(captions)
    embed_W = np.asarray(embed_W)
    W_ih = np.asarray(W_ih)
    W_hh = np.asarray(W_hh)
    b_ih = np.asarray(b_ih)
    b_hh = np.asarray(b_hh)
    fc_W = np.asarray(fc_W)
    fc_b = np.asarray(fc_b)

    if "nc" not in _NC_CACHE:
        _NC_CACHE["nc"] = build_nc()
    nc = _NC_CACHE["nc"]

    shared, per_core = prep_host(
        features, captions, embed_W, W_ih, W_hh, b_ih, b_hh, fc_W, fc_b
    )
    in_maps = [{**shared, **pc} for pc in per_core]
    res = run_bass_kernel_spmd(nc, in_maps, list(range(NCORES)))
    _NC_CACHE["last_results"] = res
    _NC_CACHE["last_in_maps"] = in_maps

    # out_lg: [NVT, 128, src_core, block, S*BC] -> out[b, t, v]
    out = np.empty((B, T, V), np.float32)
    for c in range(NCORES):
        lg = res.results[c]["out_lg"]  # vocab rows [VC*c : VC*(c+1)]
        # [v1, p, k, b, s, j] -> batch 16k+j, step 8b+s, vocab 128*v1+p
        arr = lg.reshape(NVT, 128, NCORES, NB, S, BC)
        arr = arr.transpose(2, 5, 3, 4, 0, 1).reshape(B, T, VC)
        nv = min(VC, V - c * VC)
        if nv > 0:
            out[:, :, c * VC : c * VC + nv] = arr[:, :, :nv]
    return out
